# revision 1
# baseline (speedup 1.0000x reference)
"""RNN-T joint network kernel for 8 Trainium2 NeuronCores.

out[b,t,u,:] = W2 @ tanh(W1e @ enc[b,t] + W1d @ dec[b,u] + b1) + b2

Shapes: B=4, T=200, U=100, D=512, H=1024, O=512 (all fp32).
Sharding: T split 8 ways (25 t's per core); dec + weights replicated.

Per-core device program:
  Phase 1: enc_hT[h, b*25+t] = W1e @ encT (+b1), dec_hT[h, b*100+u] = W1d @ decT
           (h on partitions in 8 chunks of 128; small matmuls).
  Phase 2: for each chunk (b, 5 t's) = 500 rows:
           s[kchunk, t, u] = dec_hT[k][:, b-block] (+) enc_hT broadcast  (DVE,
           stride-0 broadcast APs), tanh over the whole [128, 4000] tile (ACT),
           then 4x8 accumulating matmuls against W2T chunks -> psum [128, 512],
           add b2 + copy to SBUF (DVE), DMA out.
"""

from contextlib import ExitStack

import numpy as np

import concourse.bacc as bacc
import concourse.bass as bass
import concourse.mybir as mybir
import concourse.tile as tile
from concourse.bass_utils import run_bass_kernel_spmd

F32 = mybir.dt.float32
F32R = mybir.dt.float32r

B, T, U, D, H, O = 4, 200, 100, 512, 1024, 512
NCORES = 8
TLOC = T // NCORES            # 25 t's per core
PAIRS = B * TLOC              # 100 (b,t) pairs per core
TCH = 5                       # t's per inner chunk
CHROWS = TCH * U              # 500 rows per chunk
NCH = TLOC // TCH             # 5 chunks per b
ROWS = PAIRS * U              # 10000 output rows per core
DK = D // 128                 # 4 contraction chunks for phase 1
HK = H // 128                 # 8 h chunks

_CACHE = {}


def _build():
    nc = bacc.Bacc("TRN2", target_bir_lowering=False, debug=False,
                   num_devices=NCORES)
    # inputs arrive pre-interleaved in SBUF layout: [128, nchunk*width],
    # partition p holding chunk k's row (k*128+p) at cols [k*width, ...)
    encT = nc.dram_tensor("encT", [128, DK * PAIRS], F32, kind="ExternalInput")
    decT = nc.dram_tensor("decT", [128, DK * B * U], F32, kind="ExternalInput")
    w1eT = nc.dram_tensor("w1eT", [128, DK * H], F32, kind="ExternalInput")
    w1dT = nc.dram_tensor("w1dT", [128, DK * H], F32, kind="ExternalInput")
    w2T = nc.dram_tensor("w2T", [128, HK * O], F32, kind="ExternalInput")
    b1r = nc.dram_tensor("b1r", [128, HK], F32, kind="ExternalInput")
    b2c = nc.dram_tensor("b2c", [128, O // 128], F32, kind="ExternalInput")
    out = nc.dram_tensor("out", [O, ROWS], F32, kind="ExternalOutput")

    BU = B * U
    with tile.TileContext(nc) as tc, ExitStack() as ctx:
        consts = ctx.enter_context(tc.tile_pool(name="consts", bufs=1))
        spool = ctx.enter_context(tc.tile_pool(name="spool", bufs=4))
        opool = ctx.enter_context(tc.tile_pool(name="opool", bufs=8))
        psB = ctx.enter_context(tc.tile_pool(name="psB", bufs=8, space="PSUM"))

        # ---- load constants / inputs ----
        w1e_s = consts.tile([128, DK * H], F32)      # dk-chunk k at cols [k*H, (k+1)*H)
        w1d_s = consts.tile([128, DK * H], F32)
        w2_s = consts.tile([128, HK * O], F32)       # hk-chunk k at cols [k*O, (k+1)*O)
        encT_s = consts.tile([128, DK * PAIRS], F32)
        decT_s = consts.tile([128, DK * BU], F32)
        b1_s = consts.tile([128, HK], F32)
        b2c_s = consts.tile([128, O // 128], F32)
        # split loads across the two HWDGE rings (sync + scalar) so the
        # enc-side and dec-side transfers run in parallel; all plain 2D
        # contiguous DMAs (inputs are pre-interleaved on the host)
        nc.sync.dma_start(encT_s[:], encT[:])
        nc.scalar.dma_start(decT_s[:], decT[:])
        nc.sync.dma_start(w1e_s[:], w1eT[:])
        nc.scalar.dma_start(w1d_s[:], w1dT[:])
        nc.sync.dma_start(w2_s[:], w2T[:])
        nc.scalar.dma_start(b1_s[:], b1r[:])
        nc.scalar.dma_start(b2c_s[:], b2c[:])

        # float32r copies (fp32r matmul inputs must come from rounding
        # producers; DMA does not qualify)
        w1e_r = consts.tile([128, DK * H], F32R)
        w1d_r = consts.tile([128, DK * H], F32R)
        w2_r = consts.tile([128, HK * O], F32R)
        encT_r = consts.tile([128, DK * PAIRS], F32R)
        decT_r = consts.tile([128, DK * BU], F32R)
        nc.vector.tensor_copy(encT_r[:], encT_s[:])
        nc.vector.tensor_copy(w1e_r[:], w1e_s[:])
        nc.vector.tensor_copy(decT_r[:], decT_s[:])
        nc.vector.tensor_copy(w1d_r[:], w1d_s[:])
        nc.vector.tensor_copy(w2_r[:], w2_s[:])

        # ---- phase 1: enc_hT (+b1) and dec_hT ----
        # per-k tiles so phase-2 builds can start as soon as *their* k chunk
        # is ready (a single big tile would serialize phase 2 behind all of
        # phase 1 via coarse dependency tracking)
        ench_t = [consts.tile([128, PAIRS], F32, name=f"ench{k}") for k in range(HK)]
        dech_t = [consts.tile([128, BU], F32, name=f"dech{k}") for k in range(HK)]
        # enc matmuls first: they only need encT+w1e, and cover the
        # decT/w1d DMA + cast latency with PE work
        for hk in range(HK):
            pe = psB.tile([128, 512], F32, tag="psB", name="pe")
            pe = pe[:, :PAIRS]
            for dk in range(DK):
                nc.tensor.matmul(
                    pe[:],
                    lhsT=w1e_r[:, dk * H + hk * 128: dk * H + (hk + 1) * 128],
                    rhs=encT_r[:, dk * PAIRS:(dk + 1) * PAIRS],
                    start=(dk == 0), stop=(dk == DK - 1),
                )
            nc.vector.tensor_scalar_add(ench_t[hk][:], pe[:], b1_s[:, hk:hk + 1])
        for hk in range(HK):
            pd = psB.tile([128, 512], F32, tag="psB", name="pd")
            pd = pd[:, :BU]
            for dk in range(DK):
                nc.tensor.matmul(
                    pd[:],
                    lhsT=w1d_r[:, dk * H + hk * 128: dk * H + (hk + 1) * 128],
                    rhs=decT_r[:, dk * BU:(dk + 1) * BU],
                    start=(dk == 0), stop=(dk == DK - 1),
                )
            nc.vector.tensor_copy(dech_t[hk][:], pd[:])

        # ---- phase 2: chunks of (b, up to 5 t's) ----
        # small leading chunks shorten the build+tanh fill before the first
        # big matmul group
        chunks = []
        for b in range(B):
            sizes = [1, 4] + [TCH] * 4 if b == 0 else [TCH] * NCH
            t0c = 0
            for tch in sizes:
                chunks.append((b, t0c, tch))
                t0c += tch
        for b, t0c, tch in chunks:
            rows_c = tch * U
            s_t = spool.tile([128, HK * CHROWS], F32R, tag="s")
            for k in range(HK):
                in0 = dech_t[k][:, b * U:(b + 1) * U]
                in0 = in0.rearrange("p (a u) -> p a u", a=1)
                c0 = b * TLOC + t0c
                in1 = ench_t[k][:, c0:c0 + tch].rearrange("p (t a) -> p t a", a=1)
                bc0, bc1 = bass.broadcast_tensor_aps(in0, in1)
                outap = s_t[:, k * CHROWS: k * CHROWS + rows_c].rearrange(
                    "p (t u) -> p t u", t=tch)
                nc.vector.tensor_tensor(outap, bc0, bc1, mybir.AluOpType.add)
            s_used = s_t[:].rearrange("p (k c) -> p k c", k=HK)[:, :, :rows_c]
            nc.scalar.activation(s_used, s_used,
                                 mybir.ActivationFunctionType.Tanh)
            row0 = b * (TLOC * U) + t0c * U
            # swapped matmul: W2 blocks stationary, s moving -> psum holds
            # out^T [o-chunk, rows]; b2 folds into the psum->sbuf copy as a
            # per-partition bias.
            for oc in range(O // 128):
                ps = psB.tile([128, 512], F32, tag="psB")
                for k in range(HK):
                    nc.tensor.matmul(
                        ps[:, :rows_c],
                        lhsT=w2_r[:, k * O + oc * 128: k * O + (oc + 1) * 128],
                        rhs=s_t[:, k * CHROWS: k * CHROWS + rows_c],
                        start=(k == 0), stop=(k == HK - 1),
                    )
                ot = opool.tile([128, CHROWS], F32, tag="ot")
                if oc < 2:
                    nc.scalar.activation(
                        ot[:, :rows_c], ps[:, :rows_c],
                        mybir.ActivationFunctionType.Identity,
                        bias=b2c_s[:, oc:oc + 1])
                else:
                    nc.vector.tensor_scalar_add(
                        ot[:, :rows_c], ps[:, :rows_c], b2c_s[:, oc:oc + 1])
                nc.sync.dma_start(
                    out[oc * 128:(oc + 1) * 128, row0:row0 + rows_c],
                    ot[:, :rows_c])
    nc.compile()
    return nc


def kernel(enc_state, dec_state, W1, b1, W2, b2, _trace=False):
    enc_state = np.ascontiguousarray(enc_state, dtype=np.float32)
    dec_state = np.ascontiguousarray(dec_state, dtype=np.float32)
    W1 = np.asarray(W1, dtype=np.float32)
    b1 = np.asarray(b1, dtype=np.float32)
    W2 = np.asarray(W2, dtype=np.float32)
    b2 = np.asarray(b2, dtype=np.float32)

    if "nc" not in _CACHE:
        _CACHE["nc"] = _build()
    nc = _CACHE["nc"]

    def chunk128(a):
        # [n*128, w] -> [128, n*w]: partition p holds row k*128+p of chunk k
        n = a.shape[0] // 128
        return np.ascontiguousarray(
            a.reshape(n, 128, a.shape[1]).transpose(1, 0, 2).reshape(128, -1))

    decT = chunk128(dec_state.reshape(B * U, D).T)                      # [128, 4*400]
    w1eT = chunk128(W1[:, :D].T)                                        # [128, 4*H]
    w1dT = chunk128(W1[:, D:].T)                                        # [128, 4*H]
    w2T = chunk128(W2.T)                                                # [128, 8*O]
    b1r = np.ascontiguousarray(b1.reshape(HK, 128).T)                   # [128, HK]
    b2cm = np.ascontiguousarray(b2.reshape(O // 128, 128).T)            # [128, 4]

    in_maps = []
    for c in range(NCORES):
        enc_c = enc_state[:, c * TLOC:(c + 1) * TLOC, :].reshape(PAIRS, D)
        encT_c = chunk128(enc_c.T)                                      # [128, 4*100]
        in_maps.append({
            "encT": encT_c, "decT": decT, "w1eT": w1eT, "w1dT": w1dT,
            "w2T": w2T, "b1r": b1r, "b2c": b2cm,
        })

    res = run_bass_kernel_spmd(nc, in_maps, list(range(NCORES)), trace=_trace)
    out = np.empty((B, T, U, O), dtype=np.float32)
    for c in range(NCORES):
        # device output is transposed: [O, ROWS]
        out[:, c * TLOC:(c + 1) * TLOC] = (
            res.results[c]["out"].T.reshape(B, TLOC, U, O))
    if _trace:
        kernel.last_results = res
    return out



# revision 7
# speedup vs baseline: 1.1220x; 1.1220x over previous
"""RNN-T joint network kernel for 8 Trainium2 NeuronCores.

out[b,t,u,:] = W2 @ tanh(W1e @ enc[b,t] + W1d @ dec[b,u] + b1) + b2

Shapes: B=4, T=200, U=100, D=512, H=1024, O=512 (fp32 in/out).
Sharding: T split 8 ways (25 t's per core); dec + weights replicated.

All matmul inputs are bf16 (rel-err budget 2e-2; measured bf16 error
~3e-3). fp8 was measured at 3.4e-2 — over budget — so phase 2 runs
bf16 at 1 cycle/row, which is the same PE rate as fp32r but with half
the DMA/SBUF traffic and no fp32r cast instructions at startup.

Per-core device program:
  Phase 1: ench[k][h,100] = W1e@encT + b1, dech[k][h,400] = W1d@decT,
           k-chunks split into A (k=0..4) and B (k=5..7) tile groups so
           phase 2 can start on the A half early.
  Phase 2: per chunk (b, up to 5 t's -> <=500 rows):
           s = ench (+) dech broadcast-add, one fused 4D-AP
           tensor_tensor per half (DVE does A, GpSimd does B),
           in-place tanh per half (ACT), then per oc in 0..3 an
           8-matmul PSUM accumulation group against W2 (A-half groups
           first, B-half after), psum->sbuf copy (ACT/DVE/GpSimd
           round-robin), DMA out on alternating rings.
  b2 is added on the host (psum cannot be DMA'd; the copy engines
  skip the bias so GpSimd's slow Add path isn't needed).
"""

from contextlib import ExitStack

import ml_dtypes
import numpy as np

import concourse.bacc as bacc
import concourse.bass as bass
import concourse.mybir as mybir
import concourse.tile as tile
from concourse.bass_utils import run_bass_kernel_spmd

F32 = mybir.dt.float32
BF16 = mybir.dt.bfloat16

B, T, U, D, H, O = 4, 200, 100, 512, 1024, 512
NCORES = 8
TLOC = T // NCORES            # 25 t's per core
PAIRS = B * TLOC              # 100 (b,t) pairs per core
ROWS = PAIRS * U              # 10000 output rows per core
DK = D // 128                 # 4 contraction chunks for phase 1
HK = H // 128                 # 8 h chunks
KA = 5                        # k chunks in the A half (k=0..4)
KB = HK - KA                  # 3 k chunks in the B half (k=5..7)
TCH = 5                       # max t's per phase-2 chunk
CHMAX = TCH * U               # 500 rows max per chunk

_CACHE = {}


def _chunks():
    # (b, t0, tch) chunk list: ramp up at the start (pipeline fill) and
    # taper at the end (fast drain)
    sizes_by_b = [
        [1, 2, 4, 5, 5, 5, 3],
        [5] * 5,
        [5] * 5,
        [5, 5, 5, 5, 4, 1],
    ]
    out = []
    for b, sizes in enumerate(sizes_by_b):
        t0 = 0
        for tch in sizes:
            out.append((b, t0, tch))
            t0 += tch
        assert t0 == TLOC
    return out


def _build():
    nc = bacc.Bacc("TRN2", target_bir_lowering=False, debug=False,
                   num_devices=NCORES)
    BU = B * U
    # inputs pre-interleaved on host: [128, nchunk*width], partition p
    # holds row k*128+p of chunk k at cols [k*width, (k+1)*width)
    encT = nc.dram_tensor("encT", [128, DK * PAIRS], BF16, kind="ExternalInput")
    decT = nc.dram_tensor("decT", [128, DK * BU], BF16, kind="ExternalInput")
    w1eT = nc.dram_tensor("w1eT", [128, DK * H], BF16, kind="ExternalInput")
    w1dT = nc.dram_tensor("w1dT", [128, DK * H], BF16, kind="ExternalInput")
    w2T = nc.dram_tensor("w2T", [128, HK * O], BF16, kind="ExternalInput")
    b1r = nc.dram_tensor("b1r", [128, HK], F32, kind="ExternalInput")
    out = nc.dram_tensor("out", [O, ROWS], F32, kind="ExternalOutput")

    with tile.TileContext(nc) as tc, ExitStack() as ctx:
        consts = ctx.enter_context(tc.tile_pool(name="consts", bufs=1))
        spoolA = ctx.enter_context(tc.tile_pool(name="spoolA", bufs=3))
        spoolB = ctx.enter_context(tc.tile_pool(name="spoolB", bufs=3))
        opool = ctx.enter_context(tc.tile_pool(name="opool", bufs=8))
        psB = ctx.enter_context(tc.tile_pool(name="psB", bufs=8, space="PSUM"))

        w1e_s = consts.tile([128, DK * H], BF16)
        w1d_s = consts.tile([128, DK * H], BF16)
        w2_s = consts.tile([128, HK * O], BF16)
        encT_s = consts.tile([128, DK * PAIRS], BF16)
        decT_s = consts.tile([128, DK * BU], BF16)
        b1_s = consts.tile([128, HK], F32)
        # sync ring: what the enc-side matmuls need first; scalar ring:
        # dec side + W2. Weight loads split in halves so the first
        # matmuls can start before the full tensor lands.
        nc.sync.dma_start(encT_s[:], encT[:])
        nc.scalar.dma_start(decT_s[:], decT[:])
        nc.scalar.dma_start(b1_s[:], b1r[:])
        half = DK * H // 2
        nc.sync.dma_start(w1e_s[:, :half], w1eT[:, :half])
        nc.sync.dma_start(w1e_s[:, half:], w1eT[:, half:])
        nc.scalar.dma_start(w1d_s[:, :half], w1dT[:, :half])
        nc.scalar.dma_start(w1d_s[:, half:], w1dT[:, half:])
        ohalf = HK * O // 2
        nc.sync.dma_start(w2_s[:, :ohalf], w2T[:, :ohalf])
        nc.scalar.dma_start(w2_s[:, ohalf:], w2T[:, ohalf:])

        # ---- phase 1 ----
        # A/B tile split gives phase 2 per-half readiness under Tile's
        # per-tile dependency tracking.
        ench_t = {"A": consts.tile([128, KA * PAIRS], BF16, name="enchA"),
                  "B": consts.tile([128, KB * PAIRS], BF16, name="enchB")}
        dech_t = {"A": consts.tile([128, KA * BU], BF16, name="dechA"),
                  "B": consts.tile([128, KB * BU], BF16, name="dechB")}

        def halfslot(k):
            return ("A", k) if k < KA else ("B", k - KA)

        for k in range(HK):
            pe = psB.tile([128, 512], F32, tag="psB", name="pe")[:, :PAIRS]
            for dk in range(DK):
                nc.tensor.matmul(
                    pe[:],
                    lhsT=w1e_s[:, dk * H + k * 128: dk * H + (k + 1) * 128],
                    rhs=encT_s[:, dk * PAIRS:(dk + 1) * PAIRS],
                    start=(dk == 0), stop=(dk == DK - 1),
                )
            hf, kk = halfslot(k)
            nc.vector.tensor_scalar_add(
                ench_t[hf][:, kk * PAIRS:(kk + 1) * PAIRS], pe[:],
                b1_s[:, k:k + 1])
        for k in range(HK):
            pd = psB.tile([128, 512], F32, tag="psB", name="pd")[:, :BU]
            for dk in range(DK):
                nc.tensor.matmul(
                    pd[:],
                    lhsT=w1d_s[:, dk * H + k * 128: dk * H + (k + 1) * 128],
                    rhs=decT_s[:, dk * BU:(dk + 1) * BU],
                    start=(dk == 0), stop=(dk == DK - 1),
                )
            hf, kk = halfslot(k)
            dst = dech_t[hf][:, kk * BU:(kk + 1) * BU]
            if k % 2 == 0:
                nc.vector.tensor_copy(dst, pd[:])
            else:
                nc.scalar.activation(dst, pd[:],
                                     mybir.ActivationFunctionType.Copy)

        # ---- phase 2 ----
        for b, t0c, tch in _chunks():
            rows_c = tch * U
            row0 = b * (TLOC * U) + t0c * U

            s_t = {"A": spoolA.tile([128, KA * CHMAX], BF16, tag="sA",
                                    name="sA"),
                   "B": spoolB.tile([128, KB * CHMAX], BF16, tag="sB",
                                    name="sB")}
            for hf, nk, eng in (("A", KA, nc.vector), ("B", KB, nc.gpsimd)):
                # fused broadcast add over (k, t, u) with 4D APs
                dech_ap = dech_t[hf][:].rearrange(
                    "p (k bu) -> p k bu", k=nk)[:, :, b * U:(b + 1) * U]
                dech_ap = dech_ap.rearrange("p k (a u) -> p k a u", a=1)
                c0 = b * TLOC + t0c
                ench_ap = ench_t[hf][:].rearrange(
                    "p (k c) -> p k c", k=nk)[:, :, c0:c0 + tch]
                ench_ap = ench_ap.rearrange("p k (t a) -> p k t a", a=1)
                bc_d, bc_e = bass.broadcast_tensor_aps(dech_ap, ench_ap)
                outap = s_t[hf][:, :nk * rows_c].rearrange(
                    "p (k t u) -> p k t u", k=nk, t=tch)
                eng.tensor_tensor(outap, bc_d, bc_e, mybir.AluOpType.add)
                nc.scalar.activation(s_t[hf][:, :nk * rows_c],
                                     s_t[hf][:, :nk * rows_c],
                                     mybir.ActivationFunctionType.Tanh)

            # A-half accumulation groups first (start), B-half closes
            # them (stop) — lets chunk 0 begin before the B half of
            # phase 1 lands.
            ps = []
            for oc in range(O // 128):
                p = psB.tile([128, 512], F32, tag="psB", name="p")[:, :rows_c]
                ps.append(p)
                for k in range(KA):
                    nc.tensor.matmul(
                        p[:],
                        lhsT=w2_s[:, k * O + oc * 128: k * O + (oc + 1) * 128],
                        rhs=s_t["A"][:, k * rows_c:(k + 1) * rows_c],
                        start=(k == 0), stop=False,
                    )
            for oc in range(O // 128):
                for kk in range(KB):
                    k = KA + kk
                    nc.tensor.matmul(
                        ps[oc][:],
                        lhsT=w2_s[:, k * O + oc * 128: k * O + (oc + 1) * 128],
                        rhs=s_t["B"][:, kk * rows_c:(kk + 1) * rows_c],
                        start=False, stop=(kk == KB - 1),
                    )
            for oc in range(O // 128):
                ot = opool.tile([128, CHMAX], F32, tag="ot",
                                name="ot")[:, :rows_c]
                # gpsimd cannot access PSUM; split copies ACT/DVE
                if oc < 2:
                    nc.scalar.activation(ot[:], ps[oc][:],
                                         mybir.ActivationFunctionType.Copy)
                else:
                    nc.vector.tensor_copy(ot[:], ps[oc][:])
                ring = nc.sync if oc % 2 == 0 else nc.scalar
                ring.dma_start(
                    out[oc * 128:(oc + 1) * 128, row0:row0 + rows_c], ot[:])
    nc.compile()
    return nc


def _chunk128(a):
    # [n*128, w] -> [128, n*w]: partition p holds row k*128+p of chunk k
    n = a.shape[0] // 128
    return np.ascontiguousarray(
        a.reshape(n, 128, a.shape[1]).transpose(1, 0, 2).reshape(128, -1))


def _bf16(a):
    return np.ascontiguousarray(a).astype(ml_dtypes.bfloat16)


def kernel(enc_state, dec_state, W1, b1, W2, b2, _trace=False):
    enc_state = np.ascontiguousarray(enc_state, dtype=np.float32)
    dec_state = np.ascontiguousarray(dec_state, dtype=np.float32)
    W1 = np.asarray(W1, dtype=np.float32)
    b1 = np.asarray(b1, dtype=np.float32)
    W2 = np.asarray(W2, dtype=np.float32)
    b2 = np.asarray(b2, dtype=np.float32)

    if "nc" not in _CACHE:
        _CACHE["nc"] = _build()
    nc = _CACHE["nc"]

    decT = _bf16(_chunk128(dec_state.reshape(B * U, D).T))
    w1eT = _bf16(_chunk128(W1[:, :D].T))
    w1dT = _bf16(_chunk128(W1[:, D:].T))
    w2T = _bf16(_chunk128(W2.T))
    b1r = np.ascontiguousarray(b1.reshape(HK, 128).T)

    in_maps = []
    for c in range(NCORES):
        enc_c = enc_state[:, c * TLOC:(c + 1) * TLOC, :].reshape(PAIRS, D)
        in_maps.append({
            "encT": _bf16(_chunk128(enc_c.T)), "decT": decT,
            "w1eT": w1eT, "w1dT": w1dT, "w2T": w2T, "b1r": b1r,
        })

    res = run_bass_kernel_spmd(nc, in_maps, list(range(NCORES)), trace=_trace)
    out = np.empty((B, T, U, O), dtype=np.float32)
    for c in range(NCORES):
        out[:, c * TLOC:(c + 1) * TLOC] = (
            res.results[c]["out"].T.reshape(B, TLOC, U, O))
    out += b2
    if _trace:
        kernel.last_results = res
    return out
